# revision 1
# baseline (speedup 1.0000x reference)
"""GatedGCN (NewGraphReasoner) Trainium2 kernel — 8-core edge-parallel SPMD, v2.

Strategy vs v1 baseline:
  * Nodes sharded 12500/core; edges owned by dst core, sorted by dst, chunked
    128/edge-chunk aligned to 128-node tiles (segment sums core-local).
  * All persistent node/edge state kept TRANSPOSED (feature-major: x_T, e_T)
    so matmul lhsT tiles come from straight DMA loads (no DMA transposes) and
    batchnorm scale/bias become per-partition scalars fused into one
    scalar-engine activation (Relu(s*x+t)).
  * Per layer: pass-2x fuses x-update + PE transpose + next layer's node
    matmuls (packed [E|B] and [D|A] weights, 512-wide psum); ONE packed
    AllGather of the [Ex|Bx] gather table; pass-2e fuses e-update + PE
    transpose + next layer's Ce (so the edge phase does no lhsT loads at all).
  * Edge phase per 128-edge chunk: one packed indirect gather [Ex|Bx][src],
    Dx[dst] via one-hot matmul (edges dst-sorted), sigmoid, one-hot scatter
    matmul into PSUM, masked stats matmuls; meta resident in SBUF.
  * x_agg uses DVE divide; x_agg values stay resident in SBUF (xg_sb).
  * Decoder: R_e precompute overlaps bf16 AllGather of P; Q[dst] via one-hot
    matmul; per-chunk outputs accumulate in SBUF, one final DMA.
"""

import os
import sys

import numpy as np

sys.path.insert(0, "/opt/trn_rl_repo")

import ml_dtypes

H = 256
HH = 128
L = 4
N = 100000
E = 300000
BN_EPS = 1e-5
AGG_EPS = 1e-6
NCORES = 8
NP_ = 12500          # nodes per core
NPAD = 12544         # padded (98*128)
NT = 98              # node tiles per core
P = 128

DEBUG_TAPS = bool(int(os.environ.get("KERNEL_DEBUG_TAPS", "0")))
PROFILE = bool(int(os.environ.get("KERNEL_PROFILE", "0")))

_bf16 = ml_dtypes.bfloat16


# ----------------------------------------------------------------- host prep
def _host_prep(edge_index, aligned):
    """Sort edges by (owner, dst); build uniform chunk structure + metadata."""
    src = edge_index[0].astype(np.int64)
    dst = edge_index[1].astype(np.int64)
    owner = dst // NP_
    order = np.lexsort((dst, owner))
    src_s, dst_s, owner_s = src[order], dst[order], owner[order]

    # per (core, node-tile) edge counts
    dst_loc = dst_s - owner_s * NP_
    tile_of = dst_loc // P
    counts = np.zeros((NCORES, NT), dtype=np.int64)
    np.add.at(counts, (owner_s, tile_of), 1)
    K_t = np.maximum(1, np.ceil(counts.max(axis=0) / P).astype(np.int64))  # [NT]
    nch = int(K_t.sum())
    EC = nch * P
    ECP = ((nch + 3) // 4) * 4 * P    # padded to 512-edge superchunks
    WR = (nch + P - 1) // P           # mrow width in 128-blocks

    chunk_base = np.concatenate([[0], np.cumsum(K_t)[:-1]])

    mi = np.zeros((NCORES, P, nch), dtype=np.int32)       # src_pg per slot
    mdst = np.full((NCORES, P, nch), -1.0, dtype=np.float32)  # dst_rel
    mval = np.zeros((NCORES, P, nch), dtype=_bf16)        # valid
    al_t = np.zeros((NCORES, 8, ECP), dtype=_bf16)
    inv_pos = np.zeros(E, dtype=np.int64)

    core_starts = np.searchsorted(owner_s, np.arange(NCORES + 1))
    for c in range(NCORES):
        lo, hi = core_starts[c], core_starts[c + 1]
        tl, dl = tile_of[lo:hi], dst_loc[lo:hi]
        sg = src_s[lo:hi]
        tstarts = np.searchsorted(tl, np.arange(NT + 1))
        pos_in_tile = np.arange(hi - lo) - tstarts[tl]
        slot = chunk_base[tl] * P + pos_in_tile      # flat padded slot
        chv = slot // P                              # chunk id
        pv = slot - chv * P                          # pos in chunk
        r = sg // NP_
        src_pg = r * NPAD + (sg - r * NP_)
        drel = (dl - tl * P).astype(np.float32)
        mi[c][pv, chv] = src_pg
        mdst[c][pv, chv] = drel
        mval[c][pv, chv] = 1.0
        al_t[c][:, slot] = aligned[order[lo:hi]].T.astype(_bf16)
        inv_pos[lo:hi] = c * EC + slot

    nmask = np.zeros((P, NT), dtype=_bf16)
    nm_flat = np.zeros(NT * P, dtype=np.float32)
    nm_flat[:NP_] = 1.0
    nmask[:, :] = nm_flat.reshape(NT, P).T.astype(_bf16)

    perm = np.empty(E, dtype=np.int64)
    perm[order] = inv_pos
    return dict(mi=mi, mdst=mdst, mval=mval, al_t=al_t,
                nmask=nmask, nch=nch, EC=EC, ECP=ECP, WR=WR, K_t=K_t,
                perm=perm)


# ------------------------------------------------------------- device kernel
def _build_nc(nch, K_t, ECP, WR, zb):
    import concourse.bass as bass
    import concourse.mybir as mybir
    from concourse.tile import TileContext

    F32 = mybir.dt.float32
    BF16 = mybir.dt.bfloat16
    I32 = mybir.dt.int32
    AF = mybir.ActivationFunctionType
    ALU = mybir.AluOpType
    EC = nch * P
    NSC = ECP // 512                     # 512-edge superchunks

    nc = bass.Bass("TRN2", target_bir_lowering=False, debug=False,
                   num_devices=NCORES)

    def inp(name, shape, dt=F32):
        return nc.dram_tensor(name, shape, dt, kind="ExternalInput")

    # ---------------- I/O ----------------
    ht = inp("ht", [2, P, NPAD], BF16)          # h_old transposed, padded
    al_t = inp("al_t", [8, ECP], BF16)
    mi_d = inp("mi_d", [P, nch], I32)
    mdst_d = inp("mdst_d", [P, nch])
    mval_d = inp("mval_d", [P, nch], BF16)
    nmask_d = inp("nmask_d", [P, NT], BF16)
    w_fu = inp("w_fu", [2, P, 2, P])            # fusion_w[256:512] blocked
    c0c = inp("c0c", [P, 2])                    # col: ones@fw[:256]+fb
    w_ep = inp("w_ep", [8, H], BF16)            # eproj_w (lhsT, K=8)
    bepc = inp("bepc", [P, 2])                  # eproj_b cols
    w_epC = inp("w_epC", [8, H], BF16)          # eproj_w @ C0
    bepC = inp("bepC", [1, H])                  # row bias for Ce0
    wEB = inp("wEB", [L, 2, P, 2 * H])          # [E|B] packed
    ebb = inp("ebb", [L, 1, 2 * H])             # [E_b|B_b]
    wDA = inp("wDA", [L, 2, P, 2 * H])          # [D|A] packed
    dab = inp("dab", [L, 1, 2 * H])             # [D_b|A_b]
    wC = inp("wC", [L, 2, P, H])
    cbr = inp("cbr", [L, 1, H])                 # C_b row
    gxg = inp("gxg", [L, 1, H])
    gxb = inp("gxb", [L, 1, H])
    geg = inp("geg", [L, 1, H])
    geb = inp("geb", [L, 1, H])
    wPQ = inp("wPQ", [2, P, 2 * H])             # [W1a|W1b]
    w1c8 = inp("w1c8", [8, H], BF16)
    bd1 = inp("bd1", [1, H])                    # dec1_b row
    w2r = inp("w2r", [1, H])                    # dec2_w row

    out_d = nc.dram_tensor("out_d", [P, nch], F32, kind="ExternalOutput")
    taps = {}
    if DEBUG_TAPS:
        for nm in ["tap_x1", "tap_x2", "tap_x3", "tap_x4", "tap_x5"]:
            taps[nm] = nc.dram_tensor(nm, [2, P, NPAD], F32,
                                      kind="ExternalOutput")
        taps["tap_e1"] = nc.dram_tensor("tap_e1", [2, P, ECP], BF16,
                                        kind="ExternalOutput")
        taps["tap_p"] = nc.dram_tensor("tap_p", [NPAD, H], BF16,
                                       kind="ExternalOutput")
        taps["tap_q"] = nc.dram_tensor("tap_q", [NPAD, H], BF16,
                                       kind="ExternalOutput")
        taps["tap_re"] = nc.dram_tensor("tap_re", [ECP, H], BF16,
                                        kind="ExternalOutput")

    core_ids = list(range(NCORES))

    with TileContext(nc) as tc:
        import contextlib
        ctx = contextlib.ExitStack()
        with ctx:
            wp = ctx.enter_context(tc.tile_pool(name="wp", bufs=1))
            sb = ctx.enter_context(tc.tile_pool(name="sb", bufs=3))
            ps = ctx.enter_context(tc.tile_pool(name="ps", bufs=2,
                                                space="PSUM"))
            dr = ctx.enter_context(tc.tile_pool(name="dr", bufs=1,
                                                space="DRAM"))

            # ------------- persistent DRAM state -------------
            x_T = dr.tile([2, P, NPAD], F32, tag="x_T")
            e_T = dr.tile([2, P, ECP], BF16, tag="e_T")
            ce_b = dr.tile([ECP, H], BF16, tag="ce_b")
            eh_b = dr.tile([EC, H], BF16, tag="eh_b")
            gEB_loc = dr.tile([NPAD, 2 * H], BF16, tag="gEB_loc")
            gEB_ag = [dr.tile([NCORES * NPAD, 2 * H], BF16, tag=f"gag{l}",
                              name=f"gag{l}", addr_space="Shared")
                      for l in range(L)]
            gDA = dr.tile([NPAD, 2 * H], BF16, tag="gDA")
            p_loc = dr.tile([NPAD, H], BF16, tag="p_loc")
            q_loc = dr.tile([NPAD, H], BF16, tag="q_loc")
            p_ag = dr.tile([NCORES * NPAD, H], BF16, tag="p_ag",
                           addr_space="Shared")
            st_i = dr.tile([1, 4 * H], F32, tag="st_i")
            st_o_l = [dr.tile([1, 4 * H], F32, tag=f"st_o{l}",
                              name=f"st_o{l}", addr_space="Shared")
                      for l in range(L)]

            # ------------- resident SBUF -------------
            def wtile(name, src, shape, dt=BF16):
                t = wp.tile(shape, dt, tag=name, name=name)
                if len(shape) == 4:
                    for k in range(shape[1]):
                        nc.gpsimd.dma_start(out=t[:, k, :, :], in_=src[k])
                elif len(shape) == 3:
                    for k in range(shape[1]):
                        nc.gpsimd.dma_start(out=t[:, k, :], in_=src[k])
                else:
                    nc.gpsimd.dma_start(out=t[:], in_=src)
                return t

            t_wfu = wtile("t_wfu", w_fu, [P, 2, 2, P])  # [ki][kih, ko, koh]
            t_c0c = wtile("t_c0c", c0c[:], [P, 2], F32)
            t_wep = wtile("t_wep", w_ep[:], [8, H])
            t_bepc = wtile("t_bepc", bepc[:], [P, 2], F32)
            t_wepC = wtile("t_wepC", w_epC[:], [8, H])
            t_bepC = wtile("t_bepC", bepC[:], [1, H])
            t_wEB = [wtile(f"t_wEB{l}", wEB[l], [P, 2, 2 * H])
                     for l in range(L)]
            t_ebb = [wtile(f"t_ebb{l}", ebb[l], [1, 2 * H]) for l in range(L)]
            t_wDA = [wtile(f"t_wDA{l}", wDA[l], [P, 2, 2 * H])
                     for l in range(L)]
            t_dab = [wtile(f"t_dab{l}", dab[l], [1, 2 * H]) for l in range(L)]
            t_wC = [wtile(f"t_wC{l}", wC[l], [P, 2, H]) for l in range(L)]
            t_cbr = [wtile(f"t_cbr{l}", cbr[l], [1, H]) for l in range(L)]
            t_wPQ = wtile("t_wPQ", wPQ, [P, 2, 2 * H])
            t_w1c8 = wtile("t_w1c8", w1c8[:], [8, H])
            t_bd1 = wtile("t_bd1", bd1[:], [1, H])
            t_w2r = wtile("t_w2r", w2r[:], [1, H])

            mi_sb = wtile("mi_sb", mi_d[:], [P, nch], I32)
            mdst_sb = wtile("mdst_sb", mdst_d[:], [P, nch], F32)
            mval_sb = wtile("mval_sb", mval_d[:], [P, nch], BF16)
            nmask_sb = wtile("nmask_sb", nmask_d[:], [P, NT], BF16)

            xg_sb = wp.tile([P, NT * H], BF16, tag="xg_sb", name="xg_sb")
            out_sb = wp.tile([P, nch], F32, tag="out_sb", name="out_sb")

            ones1 = wp.tile([1, P], BF16, tag="ones1", name="ones1")
            nc.gpsimd.memset(ones1[:], 1.0)
            epsc = wp.tile([P, 1], F32, tag="epsc", name="epsc")
            nc.gpsimd.memset(epsc[:], AGG_EPS)
            io_i = wp.tile([P, P], I32, tag="io_i", name="io_i")
            nc.gpsimd.iota(io_i[:], pattern=[[1, P]], base=0,
                           channel_multiplier=0)
            io_ff = wp.tile([P, P], F32, tag="io_ff", name="io_ff")
            nc.vector.tensor_copy(io_ff[:], io_i[:])
            iop_i = wp.tile([P, P], I32, tag="iop_i", name="iop_i")
            nc.gpsimd.iota(iop_i[:], pattern=[[0, P]], base=0,
                           channel_multiplier=1)
            iop_f = wp.tile([P, P], F32, tag="iop_f", name="iop_f")
            nc.vector.tensor_copy(iop_f[:], iop_i[:])
            ident = wp.tile([P, P], F32, tag="ident", name="ident")
            nc.vector.tensor_tensor(out=ident[:], in0=iop_f[:], in1=io_ff[:],
                                    op=ALU.is_equal)
            ident_b = wp.tile([P, P], BF16, tag="ident_b", name="ident_b")
            nc.vector.tensor_copy(ident_b[:], ident[:])

            # w2 broadcast tile
            w2r_b = wp.tile([1, H], BF16, tag="w2r_b", name="w2r_b")
            nc.gpsimd.dma_start(out=w2r_b[:], in_=w2r[:])
            pmW = ps.tile([P, 2 * H], F32, tag="bf32", name="pmw", bufs=2)
            nc.tensor.matmul(out=pmW[:, 0:H], lhsT=ones1[:], rhs=w2r_b[:],
                             start=True, stop=True)
            w2bc = wp.tile([P, H], BF16, tag="w2bc", name="w2bc")
            nc.scalar.activation(w2bc[:], pmW[:, 0:H], AF.Copy)

            def bias_acc(pm, row_bf, stop=True):
                nc.tensor.matmul(out=pm[:], lhsT=ones1[:, :pm.shape[0]],
                                 rhs=row_bf[:], start=False, stop=stop)

            # row [1,128-slice] f32 -> col [128,1] slice of dst
            def row2col(row_ap, dst_col, tag):
                pmS = ps.tile([P, H], F32, tag="pf32", name=f"pmS_{tag}",
                              bufs=2)
                nc.tensor.transpose(pmS[:, 0:1], row_ap, ident[:1, :1])
                nc.scalar.activation(dst_col, pmS[:, 0:1], AF.Copy)

            # ---------------- Phase 0a: fusion (x0, transposed) -------------
            NSN = (NPAD + 511) // 512     # 512-node supertiles (last ragged)
            for s in range(NSN):
                n0 = s * 512
                nn = min(512, NPAD - n0)
                rh = sb.tile([P, 2, 512], BF16, tag="rh", name="rh", bufs=2)
                nc.sync.dma_start(out=rh[:, 0, :nn], in_=ht[0, :, n0:n0 + nn])
                nc.scalar.dma_start(out=rh[:, 1, :nn], in_=ht[1, :, n0:n0 + nn])
                x0t = sb.tile([P, 2, 512], F32, tag="x0t", name="x0t", bufs=2)
                for fo in range(2):
                    pm = ps.tile([P, 2 * H], F32, tag="bf32", name="pm_fu")
                    for ki in range(2):
                        nc.tensor.matmul(out=pm[:, :nn],
                                         lhsT=t_wfu[:, ki, fo, :],
                                         rhs=rh[:, ki, :nn],
                                         start=(ki == 0), stop=(ki == 1))
                    nc.scalar.activation(x0t[:, fo, :nn], pm[:, :nn], AF.Relu,
                                         bias=t_c0c[:, fo:fo + 1])
                nc.sync.dma_start(
                    out=x_T[:, :, n0:n0 + nn].rearrange("a b c -> b a c"),
                    in_=x0t[:, :, :nn])
                xb = sb.tile([P, 2, 512], BF16, tag="xb", name="xb", bufs=2)
                nc.vector.tensor_copy(xb[:, :, :nn], x0t[:, :, :nn])
                # node matmuls for layer 0 tables
                for j in range((nn + P - 1) // P):
                    r0 = n0 + j * P
                    pmG = ps.tile([P, 2 * H], F32, tag="bf32", name="pmG")
                    for h in range(2):
                        nc.tensor.matmul(out=pmG[:],
                                         lhsT=xb[:, h, j * P:(j + 1) * P],
                                         rhs=t_wEB[0][:, h, :],
                                         start=(h == 0),
                                         stop=(zb and h == 1))
                    if not zb:
                        bias_acc(pmG, t_ebb[0])
                    gt = sb.tile([P, 2 * H], BF16, tag="gt", name="gt")
                    nc.vector.tensor_copy(gt[:], pmG[:])
                    nc.scalar.dma_start(out=gEB_loc[r0:r0 + P, :], in_=gt[:])
                    pmA = ps.tile([P, 2 * H], F32, tag="bf32", name="pmA")
                    for h in range(2):
                        nc.tensor.matmul(out=pmA[:],
                                         lhsT=xb[:, h, j * P:(j + 1) * P],
                                         rhs=t_wDA[0][:, h, :],
                                         start=(h == 0),
                                         stop=(zb and h == 1))
                    if not zb:
                        bias_acc(pmA, t_dab[0])
                    dat = sb.tile([P, 2 * H], BF16, tag="dat", name="dat")
                    nc.scalar.activation(dat[:], pmA[:], AF.Copy)
                    nc.sync.dma_start(out=gDA[r0:r0 + P, :], in_=dat[:])
            if DEBUG_TAPS:
                nc.sync.dma_start(out=taps["tap_x1"][:], in_=x_T[:])

            # AllGather layer-0 tables
            nc.gpsimd.collective_compute(
                "AllGather", ALU.bypass, replica_groups=[core_ids],
                ins=[gEB_loc.opt()], outs=[gEB_ag[0].opt()])

            # ---------------- Phase 0b: eproj (e0T + Ce0) ----------------
            for s in range(NSC):
                c0 = s * 512
                alt = sb.tile([8, 512], BF16, tag="alt", name="alt")
                nc.sync.dma_start(out=alt[:], in_=al_t[:, c0:c0 + 512])
                e0t = sb.tile([P, 2, 512], BF16, tag="e0t", name="e0t", bufs=2)
                for h in range(2):
                    pm = ps.tile([P, 2 * H], F32, tag="bf32", name="pm_ep")
                    nc.tensor.matmul(out=pm[:], lhsT=t_wep[:, h * P:(h + 1) * P],
                                     rhs=alt[:], start=True, stop=True)
                    nc.scalar.activation(e0t[:, h, :], pm[:], AF.Identity,
                                         bias=t_bepc[:, h:h + 1])
                nc.sync.dma_start(
                    out=e_T[:, :, c0:c0 + 512].rearrange("a b c -> b a c"),
                    in_=e0t[:])
                cet = sb.tile([P, 4, H], BF16, tag="cet", name="cet", bufs=2)
                for j in range(4):
                    pm = ps.tile([P, 2 * H], F32, tag="bf32", name="pm_ce0")
                    nc.tensor.matmul(out=pm[:, 0:H],
                                     lhsT=alt[:, j * P:(j + 1) * P],
                                     rhs=t_wepC[:], start=True, stop=zb)
                    if not zb:
                        nc.tensor.matmul(out=pm[:, 0:H], lhsT=ones1[:],
                                         rhs=t_bepC[:], start=False,
                                         stop=True)
                    nc.vector.tensor_copy(cet[:, j, :], pm[:, 0:H])
                nc.gpsimd.dma_start(
                    out=ce_b[c0:c0 + 512, :].rearrange("(a b) c -> b a c", a=4),
                    in_=cet[:])
            if DEBUG_TAPS:
                nc.sync.dma_start(out=taps["tap_e1"][:], in_=e_T[:])

            # ---------------- Layers ----------------
            for l in range(L):
                last = (l == L - 1)

                # (a) edge phase
                stp = ps.tile([33, 2 * H], F32, tag="stp", name="stp",
                              bufs=1)
                st_x = stp[0:1, :]
                st_e = stp[32:33, :]
                g_tiles = {}

                def issue_g(i, lcur=l):
                    if i >= nch:
                        return
                    gg = sb.tile([P, 2 * H], BF16, tag="g", name="g", bufs=4)
                    nc.gpsimd.indirect_dma_start(
                        out=gg[:], out_offset=None, in_=gEB_ag[lcur][:],
                        in_offset=bass.IndirectOffsetOnAxis(
                            ap=mi_sb[:, i:i + 1], axis=0))
                    g_tiles[i] = gg

                PF = 3
                for i in range(PF):
                    issue_g(i)
                ch = 0
                for t in range(NT):
                    r0 = t * P
                    dxa = sb.tile([P, 2 * H], BF16, tag="dxa", name="dxa",
                                  bufs=2)
                    nc.sync.dma_start(out=dxa[:], in_=gDA[r0:r0 + P, :])
                    pm_seg = ps.tile([P, 2 * H], F32, tag="bf32",
                                     name="pm_seg", bufs=2)
                    for k in range(int(K_t[t])):
                        c0 = ch * P
                        issue_g(ch + PF)
                        g = g_tiles.pop(ch)
                        ce = sb.tile([P, H], BF16, tag="ce", name="ce",
                                     bufs=4)
                        nc.sync.dma_start(out=ce[:], in_=ce_b[c0:c0 + P, :])
                        # sel: one-hot [edge, node]; selT via PE transpose
                        sel = sb.tile([P, P], BF16, tag="sel", name="sel")
                        nc.vector.tensor_tensor(
                            out=sel[:],
                            in0=mdst_sb[:, ch:ch + 1].to_broadcast([P, P]),
                            in1=io_ff[:], op=ALU.is_equal)
                        pm_r = ps.tile([P, 2 * H], BF16, tag="pb16",
                                       name="pm_r")
                        nc.tensor.transpose(pm_r[:, 0:P], sel[:], ident_b[:])
                        selT = sb.tile([P, P], BF16, tag="selT", name="selT")
                        nc.vector.tensor_copy(selT[:], pm_r[:, 0:P])
                        pmD = ps.tile([P, H], F32, tag="pf32", name="pmD")
                        nc.tensor.matmul(out=pmD[:], lhsT=selT[:],
                                         rhs=dxa[:, 0:H], start=True,
                                         stop=True)
                        # e_hat
                        st2 = sb.tile([P, 2 * H], BF16, tag="st2", name="st2", bufs=2)
                        t1 = sb.tile([P, H], BF16, tag="t1", name="t1")
                        nc.vector.tensor_tensor(out=t1[:], in0=pmD[:],
                                                in1=ce[:], op=ALU.add)
                        nc.vector.tensor_tensor(out=st2[:, 0:H], in0=t1[:],
                                                in1=g[:, 0:H], op=ALU.add)
                        if not last:
                            nc.sync.dma_start(out=eh_b[c0:c0 + P, :],
                                              in_=st2[:, 0:H])
                        sg = sb.tile([P, 2 * H], BF16, tag="sg", name="sg", bufs=2)
                        nc.scalar.activation(sg[:, H:2 * H], st2[:, 0:H],
                                             AF.Sigmoid)
                        nc.gpsimd.tensor_tensor(out=sg[:, 0:H],
                                                 in0=sg[:, H:2 * H],
                                                 in1=g[:, H:2 * H],
                                                 op=ALU.mult)
                        nc.tensor.matmul(out=pm_seg[:], lhsT=sel[:], rhs=sg[:],
                                         start=(k == 0),
                                         stop=(k == int(K_t[t]) - 1))
                        if not last:
                            nc.scalar.activation(st2[:, H:2 * H],
                                                 st2[:, 0:H], AF.Square)
                            nc.tensor.matmul(out=st_e[:],
                                             lhsT=mval_sb[:, ch:ch + 1],
                                             rhs=st2[:], start=(ch == 0),
                                             stop=(ch == nch - 1),
                                             skip_group_check=True)
                        ch += 1
                    # x_agg for tile t -> xg_sb
                    lg = sb.tile([P, H], F32, tag="lg", name="lg", bufs=2)
                    nc.scalar.activation(lg[:], pm_seg[:, H:2 * H], AF.Ln,
                                         bias=epsc[:, 0:1])
                    stx = sb.tile([P, 2 * H], BF16, tag="stx", name="stx", bufs=2)
                    rc = sb.tile([P, H], F32, tag="rc", name="rc", bufs=2)
                    nc.scalar.activation(rc[:], lg[:], AF.Exp, scale=-1.0)
                    d1 = sb.tile([P, H], F32, tag="d1", name="d1", bufs=2)
                    nc.vector.tensor_tensor(out=d1[:], in0=pm_seg[:, 0:H],
                                            in1=rc[:], op=ALU.mult)
                    nc.gpsimd.tensor_tensor(out=stx[:, 0:H], in0=d1[:],
                                            in1=dxa[:, H:2 * H], op=ALU.add)
                    nc.gpsimd.tensor_copy(xg_sb[:, t * H:(t + 1) * H],
                                          stx[:, 0:H])
                    nc.vector.tensor_tensor(out=stx[:, H:2 * H],
                                            in0=stx[:, 0:H],
                                            in1=stx[:, 0:H], op=ALU.mult)
                    nc.tensor.matmul(out=st_x[:], lhsT=nmask_sb[:, t:t + 1],
                                     rhs=stx[:], start=(t == 0),
                                     stop=(t == NT - 1),
                                     skip_group_check=True)

                # (b) stats AllReduce
                stc = sb.tile([1, 4 * H], F32, tag="stc", name="stc", bufs=1)
                nc.vector.tensor_copy(stc[:, 0:2 * H], st_x[:])
                if not last:
                    nc.vector.tensor_copy(stc[:, 2 * H:4 * H], st_e[:])
                else:
                    nc.gpsimd.memset(stc[:, 2 * H:4 * H], 0.0)
                nc.sync.dma_start(out=st_i[:], in_=stc[:])
                nc.gpsimd.collective_compute(
                    "AllReduce", ALU.add, replica_groups=[core_ids],
                    ins=[st_i.opt()], outs=[st_o_l[l].opt()])
                stg = sb.tile([1, 4 * H], F32, tag="stg", name="stg", bufs=1)
                nc.sync.dma_start(out=stg[:], in_=st_o_l[l][:])

                # (c) BN s,t as per-partition columns
                def bn_cols(sl, cnt, g_ap, b_ap, nm_):
                    mu = sb.tile([1, H], F32, tag="mu", name=f"mu{nm_}",
                                 bufs=1)
                    nc.scalar.mul(mu[:], stg[:, sl:sl + H], 1.0 / cnt)
                    m2 = sb.tile([1, H], F32, tag="m2", name=f"m2{nm_}",
                                 bufs=1)
                    nc.scalar.mul(m2[:], stg[:, sl + H:sl + 2 * H], 1.0 / cnt)
                    var = sb.tile([1, H], F32, tag="var",
                                  name=f"var{nm_}", bufs=1)
                    nc.vector.tensor_tensor(out=var[:], in0=mu[:], in1=mu[:],
                                            op=ALU.mult)
                    nc.vector.tensor_tensor(out=var[:], in0=m2[:], in1=var[:],
                                            op=ALU.subtract)
                    nc.vector.tensor_scalar_add(var[:], var[:], BN_EPS)
                    sd = sb.tile([1, H], F32, tag="sd", name=f"sd{nm_}",
                                 bufs=1)
                    nc.scalar.activation(sd[:], var[:], AF.Sqrt)
                    rs = sb.tile([1, H], F32, tag="rs", name=f"rs{nm_}",
                                 bufs=1)
                    nc.vector.reciprocal(rs[:], sd[:])
                    gg = sb.tile([1, H], F32, tag="gg", name=f"gg{nm_}",
                                 bufs=1)
                    nc.sync.dma_start(out=gg[:], in_=g_ap)
                    bb = sb.tile([1, H], F32, tag="bb", name=f"bb{nm_}",
                                 bufs=1)
                    nc.scalar.dma_start(out=bb[:], in_=b_ap)
                    srow = sb.tile([1, H], F32, tag="sr",
                                   name=f"sr{nm_}", bufs=1)
                    nc.vector.tensor_tensor(out=srow[:], in0=gg[:], in1=rs[:],
                                            op=ALU.mult)
                    trow = sb.tile([1, H], F32, tag="tr",
                                   name=f"tr{nm_}", bufs=1)
                    nc.vector.tensor_tensor(out=trow[:], in0=mu[:],
                                            in1=srow[:], op=ALU.mult)
                    nc.vector.tensor_tensor(out=trow[:], in0=bb[:],
                                            in1=trow[:], op=ALU.subtract)
                    scol = sb.tile([P, 2], F32, tag=f"sc{nm_}",
                                   name=f"sc{nm_}", bufs=1)
                    tcol = sb.tile([P, 2], F32, tag=f"tc{nm_}",
                                   name=f"tc{nm_}", bufs=1)
                    for h in range(2):
                        row2col(srow[:, h * P:(h + 1) * P], scol[:, h:h + 1],
                                f"s{nm_}{h}")
                        row2col(trow[:, h * P:(h + 1) * P], tcol[:, h:h + 1],
                                f"t{nm_}{h}")
                    return scol, tcol

                sxc, txc = bn_cols(0, N, gxg[l], gxb[l], "x")
                if not last:
                    sec, tec = bn_cols(2 * H, E, geg[l], geb[l], "e")

                # (d) pass-2 x fused with next-layer node matmuls
                for t in range(NT):
                    r0 = t * P
                    pmT = ps.tile([P, 2 * H], BF16, tag="pb16", name="pmT")
                    for h in range(2):
                        nc.tensor.transpose(
                            pmT[:, h * P:(h + 1) * P],
                            xg_sb[:, t * H + h * P:t * H + (h + 1) * P],
                            ident_b[:])
                    xbn = sb.tile([P, 2, P], BF16, tag="xbn", name="xbn")
                    for h in range(2):
                        nc.scalar.activation(xbn[:, h, :],
                                             pmT[:, h * P:(h + 1) * P],
                                             AF.Relu, bias=txc[:, h:h + 1],
                                             scale=sxc[:, h:h + 1])
                    xoT = sb.tile([P, 2, P], F32, tag="xoT", name="xoT")
                    nc.scalar.dma_start(
                        out=xoT[:],
                        in_=x_T[:, :, r0:r0 + P].rearrange("a b c -> b a c"))
                    xnT = sb.tile([P, 2, P], F32, tag="xnT", name="xnT")
                    nc.vector.tensor_tensor(out=xnT[:], in0=xoT[:],
                                            in1=xbn[:], op=ALU.add)
                    if not last or DEBUG_TAPS:
                        nc.sync.dma_start(
                            out=x_T[:, :, r0:r0 + P].rearrange(
                                "a b c -> b a c"),
                            in_=xnT[:])
                    lhx = sb.tile([P, 2, P], BF16, tag="lhx", name="lhx")
                    nc.vector.tensor_copy(lhx[:], xnT[:])
                    if not last:
                        pmG = ps.tile([P, 2 * H], F32, tag="bf32", name="pmG2")
                        for h in range(2):
                            nc.tensor.matmul(out=pmG[:], lhsT=lhx[:, h, :],
                                             rhs=t_wEB[l + 1][:, h, :],
                                             start=(h == 0),
                                             stop=(zb and h == 1))
                        if not zb:
                            bias_acc(pmG, t_ebb[l + 1])
                        gt = sb.tile([P, 2 * H], BF16, tag="gt", name="gt2")
                        nc.vector.tensor_copy(gt[:], pmG[:])
                        nc.gpsimd.dma_start(out=gEB_loc[r0:r0 + P, :],
                                            in_=gt[:])
                        pmA = ps.tile([P, 2 * H], F32, tag="bf32", name="pmA2")
                        for h in range(2):
                            nc.tensor.matmul(out=pmA[:], lhsT=lhx[:, h, :],
                                             rhs=t_wDA[l + 1][:, h, :],
                                             start=(h == 0),
                                             stop=(zb and h == 1))
                        if not zb:
                            bias_acc(pmA, t_dab[l + 1])
                        dat = sb.tile([P, 2 * H], BF16, tag="dat",
                                      name="dat2")
                        nc.scalar.activation(dat[:], pmA[:], AF.Copy)
                        nc.sync.dma_start(out=gDA[r0:r0 + P, :], in_=dat[:])
                    else:
                        pmG = ps.tile([P, 2 * H], F32, tag="bf32", name="pmPQ")
                        for h in range(2):
                            nc.tensor.matmul(out=pmG[:], lhsT=lhx[:, h, :],
                                             rhs=t_wPQ[:, h, :],
                                             start=(h == 0), stop=(h == 1))
                        gt = sb.tile([P, 2 * H], BF16, tag="gt", name="gtPQ")
                        nc.vector.tensor_copy(gt[:], pmG[:])
                        nc.scalar.dma_start(out=p_loc[r0:r0 + P, :],
                                            in_=gt[:, 0:H])
                        nc.sync.dma_start(out=q_loc[r0:r0 + P, :],
                                          in_=gt[:, H:2 * H])
                if DEBUG_TAPS:
                    nc.sync.dma_start(out=taps[f"tap_x{l + 2}"][:], in_=x_T[:])

                # (e) AllGather next tables (overlaps pass-2 e)
                if not last:
                    nc.gpsimd.collective_compute(
                        "AllGather", ALU.bypass, replica_groups=[core_ids],
                        ins=[gEB_loc.opt()], outs=[gEB_ag[l + 1].opt()])
                else:
                    nc.gpsimd.collective_compute(
                        "AllGather", ALU.bypass, replica_groups=[core_ids],
                        ins=[p_loc.opt()], outs=[p_ag.opt()])

                # (f) pass-2 e fused with next-layer Ce
                if not last:
                    u = 0
                    while u * 2 < nch:
                        w = min(2, nch - u * 2)
                        c0 = u * 2 * P
                        ww = w * P
                        ea = sb.tile([P, 2, H], BF16, tag="ea", name="ea", bufs=2)
                        nc.sync.dma_start(
                            out=ea[:, :w, :],
                            in_=eh_b[c0:c0 + ww, :].rearrange(
                                "(a b) c -> b a c", a=w))
                        pmT2 = ps.tile([P, 2 * H], BF16, tag="pb16",
                                       name="pmT2")
                        for h in range(2):
                            for j in range(w):
                                nc.tensor.transpose(
                                    pmT2[:, h * ww + j * P:h * ww + (j + 1) * P],
                                    ea[:, j, h * P:(h + 1) * P], ident_b[:])
                        ebn = sb.tile([P, 2, 2 * P], BF16, tag="ebn",
                                      name="ebn")
                        for h in range(2):
                            nc.scalar.activation(ebn[:, h, :ww],
                                                 pmT2[:, h * ww:h * ww + ww],
                                                 AF.Relu,
                                                 bias=tec[:, h:h + 1],
                                                 scale=sec[:, h:h + 1])
                        eoT = sb.tile([P, 2, 2 * P], BF16, tag="eoT",
                                      name="eoT", bufs=2)
                        nc.scalar.dma_start(
                            out=eoT[:, :, :ww],
                            in_=e_T[:, :, c0:c0 + ww].rearrange(
                                "a b c -> b a c"))
                        enT = sb.tile([P, 2, 2 * P], BF16, tag="enT",
                                      name="enT")
                        nc.vector.tensor_tensor(out=enT[:, :, :ww],
                                                in0=eoT[:, :, :ww],
                                                in1=ebn[:, :, :ww],
                                                op=ALU.add)
                        nc.sync.dma_start(
                            out=e_T[:, :, c0:c0 + ww].rearrange(
                                "a b c -> b a c"),
                            in_=enT[:, :, :ww])
                        cet = sb.tile([P, 2, H], BF16, tag="cet2",
                                      name="cet2")
                        for j in range(w):
                            pm = ps.tile([P, 2 * H], F32, tag="bf32",
                                         name="pmCe")
                            for h in range(2):
                                nc.tensor.matmul(out=pm[:, 0:H],
                                                 lhsT=enT[:, h,
                                                          j * P:(j + 1) * P],
                                                 rhs=t_wC[l + 1][:, h, :],
                                                 start=(h == 0),
                                                 stop=(zb and h == 1))
                            if not zb:
                                nc.tensor.matmul(out=pm[:, 0:H],
                                                 lhsT=ones1[:],
                                                 rhs=t_cbr[l + 1][:],
                                                 start=False, stop=True)
                            nc.vector.tensor_copy(cet[:, j, :], pm[:, 0:H])
                        nc.gpsimd.dma_start(
                            out=ce_b[c0:c0 + ww, :].rearrange(
                                "(a b) c -> b a c", a=w),
                            in_=cet[:, :w, :])
                        u += 1

            # ---------------- Decoder ----------------
            # R_e = aligned @ W1c + dec1_b  (overlaps AllGather of P)
            for s in range(NSC):
                c0 = s * 512
                alt = sb.tile([8, 512], BF16, tag="alt", name="alt_d")
                nc.sync.dma_start(out=alt[:], in_=al_t[:, c0:c0 + 512])
                ret = sb.tile([P, 4, H], BF16, tag="ret", name="ret", bufs=2)
                for j in range(4):
                    pm = ps.tile([P, 2 * H], F32, tag="bf32", name="pm_re")
                    nc.tensor.matmul(out=pm[:, 0:H],
                                     lhsT=alt[:, j * P:(j + 1) * P],
                                     rhs=t_w1c8[:], start=True, stop=zb)
                    if not zb:
                        nc.tensor.matmul(out=pm[:, 0:H], lhsT=ones1[:],
                                         rhs=t_bd1[:], start=False, stop=True)
                    nc.vector.tensor_copy(ret[:, j, :], pm[:, 0:H])
                nc.gpsimd.dma_start(
                    out=ce_b[c0:c0 + 512, :].rearrange("(a b) c -> b a c",
                                                       a=4),
                    in_=ret[:])

            if DEBUG_TAPS:
                nc.sync.dma_start(out=taps["tap_p"][:], in_=p_loc[:])
                nc.sync.dma_start(out=taps["tap_q"][:], in_=q_loc[:])
                nc.sync.dma_start(out=taps["tap_re"][:], in_=ce_b[:])
            pg_tiles = {}

            def issue_pg(i):
                if i >= nch:
                    return
                gg = sb.tile([P, H], BF16, tag="pg", name="pg", bufs=4)
                nc.gpsimd.indirect_dma_start(
                    out=gg[:], out_offset=None, in_=p_ag[:],
                    in_offset=bass.IndirectOffsetOnAxis(
                        ap=mi_sb[:, i:i + 1], axis=0))
                pg_tiles[i] = gg

            PF = 3
            for i in range(PF):
                issue_pg(i)
            ch = 0
            for t in range(NT):
                r0 = t * P
                qt = sb.tile([P, H], BF16, tag="qt", name="qt", bufs=2)
                nc.sync.dma_start(out=qt[:], in_=q_loc[r0:r0 + P, :])
                for k in range(int(K_t[t])):
                    c0 = ch * P
                    issue_pg(ch + PF)
                    pg = pg_tiles.pop(ch)
                    re = sb.tile([P, H], BF16, tag="re", name="re", bufs=4)
                    nc.scalar.dma_start(out=re[:], in_=ce_b[c0:c0 + P, :])
                    seld = sb.tile([P, P], BF16, tag="sel", name="seld")
                    nc.vector.tensor_tensor(
                        out=seld[:],
                        in0=mdst_sb[:, ch:ch + 1].to_broadcast([P, P]),
                        in1=io_ff[:], op=ALU.is_equal)
                    pm_r = ps.tile([P, 2 * H], BF16, tag="pb16",
                                   name="pm_rd")
                    nc.tensor.transpose(pm_r[:, 0:P], seld[:], ident_b[:])
                    selT = sb.tile([P, P], BF16, tag="selT", name="selTd")
                    nc.vector.tensor_copy(selT[:], pm_r[:, 0:P])
                    pmQ = ps.tile([P, H], F32, tag="pf32", name="pmQ")
                    nc.tensor.matmul(out=pmQ[:], lhsT=selT[:], rhs=qt[:],
                                     start=True, stop=True)
                    h1 = sb.tile([P, H], BF16, tag="h1", name="h1")
                    nc.gpsimd.tensor_tensor(out=h1[:], in0=pg[:], in1=re[:],
                                            op=ALU.add)
                    h2 = sb.tile([P, H], F32, tag="h2", name="h2")
                    nc.vector.tensor_tensor(out=h2[:], in0=pmQ[:], in1=h1[:],
                                            op=ALU.add)
                    nc.scalar.activation(h2[:], h2[:], AF.Relu)
                    ov = sb.tile([P, H], F32, tag="ov", name="ov")
                    nc.vector.tensor_tensor(out=ov[:], in0=h2[:],
                                            in1=w2bc[:], op=ALU.mult)
                    nc.vector.tensor_reduce(out=out_sb[:, ch:ch + 1],
                                            in_=ov[:], op=ALU.add,
                                            axis=mybir.AxisListType.X)
                    ch += 1
            nc.sync.dma_start(out=out_d[:], in_=out_sb[:])

    _split_excess_waits(nc, mybir)
    return nc


def _split_excess_waits(nc, mybir, max_waits=1):
    """walrus in this env accepts max 1 sem wait per instruction: spill
    extras onto same-engine nops placed before the instruction."""
    for f in nc.m.functions:
        for bb in f.blocks:
            insts = list(bb.instructions)
            out_l = []
            for inst in insts:
                si = inst.sync_info
                waits = list(si.on_wait) if (si and si.on_wait) else []
                if len(waits) > max_waits:
                    extra = waits[:-max_waits]
                    keep = waits[-max_waits:]
                    for i in range(0, len(extra), max_waits):
                        nop = mybir.InstNoOp(
                            name=nc.get_next_instruction_name(),
                            engine=inst.engine, ins=[], outs=[],
                            sync_info=mybir.SyncInfo(
                                on_wait=extra[i:i + max_waits], on_update=[]))
                        nc.register_instruction(nop)
                        out_l.append(nop)
                    del si.on_wait[:]
                    si.on_wait.extend(keep)
                out_l.append(inst)
            if len(out_l) != len(insts):
                bb.instructions = out_l


# ----------------------------------------------------------------- wrapper
_CACHE = {}


def kernel(**inputs):
    edge_index = np.asarray(inputs["edge_index_new"])
    aligned = np.asarray(inputs["aligned_features"], dtype=np.float32)
    h_old = np.asarray(inputs["h_nodes_old"], dtype=np.float32)
    assert int(inputs["num_nodes"]) == N

    prep = _host_prep(edge_index, aligned)
    nch, EC, ECP, WR = prep["nch"], prep["EC"], prep["ECP"], prep["WR"]

    fw = np.asarray(inputs["fusion_w"], np.float32)
    fb = np.asarray(inputs["fusion_b"], np.float32)
    c0 = fw[:H].sum(axis=0) + fb                     # [256]

    def g(nm):
        return np.asarray(inputs[nm], np.float32)

    wEB = np.stack([np.concatenate([g("E_w")[l], g("B_w")[l]], axis=1)
                    .reshape(2, P, 2 * H) for l in range(L)])
    ebb = np.stack([np.concatenate([g("E_b")[l], g("B_b")[l]])[None]
                    for l in range(L)])
    wDA = np.stack([np.concatenate([g("D_w")[l], g("A_w")[l]], axis=1)
                    .reshape(2, P, 2 * H) for l in range(L)])
    dab = np.stack([np.concatenate([g("D_b")[l], g("A_b")[l]])[None]
                    for l in range(L)])
    wC = np.stack([g("C_w")[l].reshape(2, P, H) for l in range(L)])
    cbr = np.stack([g("C_b")[l][None] for l in range(L)])

    ep_w = g("eproj_w")                              # [8,256]
    ep_b = g("eproj_b")
    w_epC = ep_w @ g("C_w")[0]                       # [8,256]
    bepC = (ep_b @ g("C_w")[0] + g("C_b")[0])[None]

    d1 = g("dec1_w")                                 # [520,256]
    d2 = g("dec2_w")                                 # [256,1]

    # fusion weights blocked: [ki, kih(128), ko, koh(128)]
    wfu = fw[H:].reshape(2, P, 2, P)

    zb = all(np.abs(g(nm)).max() == 0 for nm in
             ["A_b", "B_b", "C_b", "D_b", "E_b", "dec1_b"])
    key = (nch, zb) + tuple(prep["K_t"])
    if key not in _CACHE:
        _CACHE[key] = _build_nc(nch, prep["K_t"], ECP, WR, zb)
    nc = _CACHE[key]

    shared = {
        "w_fu": wfu, "c0c": c0.reshape(2, P).T,
        "w_ep": ep_w.astype(_bf16), "bepc": ep_b.reshape(2, P).T,
        "w_epC": w_epC.astype(_bf16), "bepC": bepC,
        "wEB": wEB, "ebb": ebb, "wDA": wDA, "dab": dab,
        "wC": wC, "cbr": cbr,
        "gxg": g("bn_x_g")[:, None, :], "gxb": g("bn_x_b")[:, None, :],
        "geg": g("bn_e_g")[:, None, :], "geb": g("bn_e_b")[:, None, :],
        "wPQ": np.concatenate([d1[:H], d1[H:2 * H]], axis=1).reshape(2, P, 2 * H),
        "w1c8": d1[2 * H:].astype(_bf16),
        "bd1": np.asarray(inputs["dec1_b"], np.float32)[None],
        "w2r": d2[:, 0][None],
        "nmask_d": prep["nmask"],
    }
    in_maps = []
    for c in range(NCORES):
        lo = c * NP_
        hT = np.zeros((2, P, NPAD), dtype=_bf16)
        hs = h_old[lo:lo + NP_].astype(_bf16)        # [12500, 256]
        hT[0, :, :NP_] = hs[:, :P].T
        hT[1, :, :NP_] = hs[:, P:].T
        m = dict(shared)
        m["ht"] = hT
        m["al_t"] = prep["al_t"][c]
        m["mi_d"] = prep["mi"][c]
        m["mdst_d"] = prep["mdst"][c]
        m["mval_d"] = prep["mval"][c]
        in_maps.append(m)

    from concourse.bass_utils import run_bass_kernel_spmd
    res = run_bass_kernel_spmd(nc, in_maps, list(range(NCORES)),
                               trace=PROFILE)
    if PROFILE and res.exec_time_ns is not None:
        print(f"HW exec time: {res.exec_time_ns} ns")

    allout = np.concatenate([np.asarray(res.results[c]["out_d"]).T.ravel()
                             for c in range(NCORES)])
    b2 = float(np.asarray(inputs["dec2_b"], np.float32).ravel()[0])
    flow = (allout[prep["perm"]] + b2).astype(np.float32)[:, None]
    if DEBUG_TAPS:
        kernel.taps = [
            {k: np.asarray(v) for k, v in r.items() if k.startswith("tap")}
            for r in res.results]
        kernel.prep = prep
    return flow



# revision 12
# speedup vs baseline: 1.2761x; 1.2761x over previous
"""GatedGCN (NewGraphReasoner) Trainium2 kernel — 8-core edge-parallel SPMD, v3.

v3 over v2 baseline (11.26ms):
  * Edge loop processes 512-edge superchunks (4 chunks): ONE batched
    indirect gather per superchunk (SWDGE fixed cost amortized 4x), batched
    elementwise chain (adds/sigmoid/mult/square on [128, 4*256] tiles).
  * selT host-precomputed (no per-chunk PE transpose + DVE copy); sel built
    by one batched is_eq.
  * x_agg uses DVE divide (no Ln/Exp -> zero activation-table thrash;
    sigmoid/square/relu/copy share one table).
  * Matmul issue order software-pipelined (pmD of superchunk s+1 issued
    before scatter of s) so the PE doesn't stall on the elementwise chain.
  * Decoder: batched gathers, adds on DVE (GpSimd only gathers), fused
    multiply+reduce via tensor_tensor_reduce.
"""

import os
import sys

import numpy as np

sys.path.insert(0, "/opt/trn_rl_repo")

import ml_dtypes

H = 256
HH = 128
L = 4
N = 100000
E = 300000
BN_EPS = 1e-5
AGG_EPS = 1e-6
NCORES = 8
NP_ = 12500          # nodes per core
NPAD = 12544         # padded (98*128)
NT = 98              # node tiles per core
P = 128

DEBUG_TAPS = bool(int(os.environ.get("KERNEL_DEBUG_TAPS", "0")))
PROFILE = bool(int(os.environ.get("KERNEL_PROFILE", "0")))

_bf16 = ml_dtypes.bfloat16


# ----------------------------------------------------------------- host prep
def _host_prep(edge_index, aligned):
    """Sort edges by (owner, dst); build uniform chunk structure + metadata."""
    src = edge_index[0].astype(np.int64)
    dst = edge_index[1].astype(np.int64)
    owner = dst // NP_
    order = np.lexsort((dst, owner))
    src_s, dst_s, owner_s = src[order], dst[order], owner[order]

    # per (core, node-tile) edge counts
    dst_loc = dst_s - owner_s * NP_
    tile_of = dst_loc // P
    counts = np.zeros((NCORES, NT), dtype=np.int64)
    np.add.at(counts, (owner_s, tile_of), 1)
    K_t = np.maximum(1, np.ceil(counts.max(axis=0) / P).astype(np.int64))  # [NT]
    nch = int(K_t.sum())
    EC = nch * P
    nch4 = ((nch + 3) // 4) * 4
    ECP = nch4 * P                    # padded to 512-edge superchunks
    WR = (nch + P - 1) // P           # mrow width in 128-blocks

    chunk_base = np.concatenate([[0], np.cumsum(K_t)[:-1]])

    mi = np.zeros((NCORES, P, nch4), dtype=np.int32)       # src_pg per slot
    mdst = np.full((NCORES, P, nch4), -1.0, dtype=np.float32)  # dst_rel
    mval = np.zeros((NCORES, P, nch4), dtype=_bf16)        # valid
    al_t = np.zeros((NCORES, 8, ECP), dtype=_bf16)
    inv_pos = np.zeros(E, dtype=np.int64)

    core_starts = np.searchsorted(owner_s, np.arange(NCORES + 1))
    for c in range(NCORES):
        lo, hi = core_starts[c], core_starts[c + 1]
        tl, dl = tile_of[lo:hi], dst_loc[lo:hi]
        sg = src_s[lo:hi]
        tstarts = np.searchsorted(tl, np.arange(NT + 1))
        pos_in_tile = np.arange(hi - lo) - tstarts[tl]
        slot = chunk_base[tl] * P + pos_in_tile      # flat padded slot
        chv = slot // P                              # chunk id
        pv = slot - chv * P                          # pos in chunk
        r = sg // NP_
        src_pg = r * NPAD + (sg - r * NP_)
        drel = (dl - tl * P).astype(np.float32)
        mi[c][pv, chv] = src_pg
        mdst[c][pv, chv] = drel
        mval[c][pv, chv] = 1.0
        al_t[c][:, slot] = aligned[order[lo:hi]].T.astype(_bf16)
        inv_pos[lo:hi] = c * EC + slot

    # host-precomputed transposed one-hot: selT[p_node, ch, q_edge]
    # = 1 iff mdst[q_edge, ch] == p_node   (padding chunks/slots -> 0)
    selT = np.zeros((NCORES, P, nch4, P), dtype=_bf16)
    pr = np.arange(P, dtype=np.float32)
    for c in range(NCORES):
        selT[c] = (pr[:, None, None] ==
                   mdst[c].T[None, :, :]).astype(_bf16)

    nmask = np.zeros((P, NT), dtype=_bf16)
    nm_flat = np.zeros(NT * P, dtype=np.float32)
    nm_flat[:NP_] = 1.0
    nmask[:, :] = nm_flat.reshape(NT, P).T.astype(_bf16)

    perm = np.empty(E, dtype=np.int64)
    perm[order] = inv_pos
    return dict(mi=mi, mdst=mdst, mval=mval, al_t=al_t,
                selT=selT.reshape(NCORES, P, nch4 * P),
                nmask=nmask, nch=nch, nch4=nch4, EC=EC, ECP=ECP, WR=WR,
                K_t=K_t, perm=perm)


# ------------------------------------------------------------- device kernel
def _build_nc(nch, K_t, ECP, WR, zb):
    import concourse.bass as bass
    import concourse.mybir as mybir
    from concourse.tile import TileContext

    F32 = mybir.dt.float32
    BF16 = mybir.dt.bfloat16
    I32 = mybir.dt.int32
    AF = mybir.ActivationFunctionType
    ALU = mybir.AluOpType
    EC = nch * P
    nch4 = ECP // P
    NS4 = nch4 // 4                      # 512-edge superchunks
    NSC = NS4
    # chunk -> (tile, k, K) schedule
    sched = []
    for t in range(NT):
        for k in range(int(K_t[t])):
            sched.append((t, k, int(K_t[t])))
    assert len(sched) == nch

    nc = bass.Bass("TRN2", target_bir_lowering=False, debug=False,
                   num_devices=NCORES)

    def inp(name, shape, dt=F32):
        return nc.dram_tensor(name, shape, dt, kind="ExternalInput")

    # ---------------- I/O ----------------
    ht = inp("ht", [2, P, NPAD], BF16)          # h_old transposed, padded
    al_t = inp("al_t", [8, ECP], BF16)
    mi_d = inp("mi_d", [P, nch4], I32)
    mdst_d = inp("mdst_d", [P, nch4])
    mval_d = inp("mval_d", [P, nch4], BF16)
    selT_d = inp("selT_d", [P, nch4 * P], BF16)
    nmask_d = inp("nmask_d", [P, NT], BF16)
    w_fu = inp("w_fu", [2, P, 2, P])            # fusion_w[256:512] blocked
    c0c = inp("c0c", [P, 2])                    # col: ones@fw[:256]+fb
    w_ep = inp("w_ep", [8, H], BF16)            # eproj_w (lhsT, K=8)
    bepc = inp("bepc", [P, 2])                  # eproj_b cols
    w_epC = inp("w_epC", [8, H], BF16)          # eproj_w @ C0
    bepC = inp("bepC", [1, H])                  # row bias for Ce0
    wEB = inp("wEB", [L, 2, P, 2 * H])          # [E|B] packed
    ebb = inp("ebb", [L, 1, 2 * H])             # [E_b|B_b]
    wDA = inp("wDA", [L, 2, P, 2 * H])          # [D|A] packed
    dab = inp("dab", [L, 1, 2 * H])             # [D_b|A_b]
    wC = inp("wC", [L, 2, P, H])
    cbr = inp("cbr", [L, 1, H])                 # C_b row
    gxg = inp("gxg", [L, 1, H])
    gxb = inp("gxb", [L, 1, H])
    geg = inp("geg", [L, 1, H])
    geb = inp("geb", [L, 1, H])
    wPQ = inp("wPQ", [2, P, 2 * H])             # [W1a|W1b]
    w1c8 = inp("w1c8", [8, H], BF16)
    bd1 = inp("bd1", [1, H])                    # dec1_b row
    w2r = inp("w2r", [1, H])                    # dec2_w row

    out_d = nc.dram_tensor("out_d", [P, nch], F32, kind="ExternalOutput")
    taps = {}
    if DEBUG_TAPS:
        for nm in ["tap_x1", "tap_x2", "tap_x3", "tap_x4", "tap_x5"]:
            taps[nm] = nc.dram_tensor(nm, [2, P, NPAD], F32,
                                      kind="ExternalOutput")
        taps["tap_e1"] = nc.dram_tensor("tap_e1", [2, P, ECP], BF16,
                                        kind="ExternalOutput")
        taps["tap_p"] = nc.dram_tensor("tap_p", [NPAD, H], BF16,
                                       kind="ExternalOutput")
        taps["tap_q"] = nc.dram_tensor("tap_q", [NPAD, H], BF16,
                                       kind="ExternalOutput")
        taps["tap_re"] = nc.dram_tensor("tap_re", [ECP, H], BF16,
                                        kind="ExternalOutput")

    core_ids = list(range(NCORES))

    with TileContext(nc) as tc:
        import contextlib
        ctx = contextlib.ExitStack()
        with ctx:
            wp = ctx.enter_context(tc.tile_pool(name="wp", bufs=1))
            sb = ctx.enter_context(tc.tile_pool(name="sb", bufs=3))
            ps = ctx.enter_context(tc.tile_pool(name="ps", bufs=2,
                                                space="PSUM"))
            dr = ctx.enter_context(tc.tile_pool(name="dr", bufs=1,
                                                space="DRAM"))

            # ------------- persistent DRAM state -------------
            x_T = dr.tile([2, P, NPAD], F32, tag="x_T")
            e_T = dr.tile([2, P, ECP], BF16, tag="e_T")
            ce_b = dr.tile([ECP, H], BF16, tag="ce_b")
            eh_b = dr.tile([ECP, H], BF16, tag="eh_b")
            gEB_loc = dr.tile([NPAD, 2 * H], BF16, tag="gEB_loc")
            gEB_ag = [dr.tile([NCORES * NPAD, 2 * H], BF16, tag=f"gag{l}",
                              name=f"gag{l}", addr_space="Shared")
                      for l in range(L)]
            gDA = dr.tile([NPAD, 2 * H], BF16, tag="gDA")
            p_loc = dr.tile([NPAD, H], BF16, tag="p_loc")
            q_loc = dr.tile([NPAD, H], BF16, tag="q_loc")
            p_ag = dr.tile([NCORES * NPAD, H], BF16, tag="p_ag",
                           addr_space="Shared")
            st_i = dr.tile([1, 4 * H], F32, tag="st_i")
            st_o_l = [dr.tile([1, 4 * H], F32, tag=f"st_o{l}",
                              name=f"st_o{l}", addr_space="Shared")
                      for l in range(L)]

            # ------------- resident SBUF -------------
            def wtile(name, src, shape, dt=BF16):
                t = wp.tile(shape, dt, tag=name, name=name)
                if len(shape) == 4:
                    for k in range(shape[1]):
                        nc.gpsimd.dma_start(out=t[:, k, :, :], in_=src[k])
                elif len(shape) == 3:
                    for k in range(shape[1]):
                        nc.gpsimd.dma_start(out=t[:, k, :], in_=src[k])
                else:
                    nc.gpsimd.dma_start(out=t[:], in_=src)
                return t

            t_wfu = wtile("t_wfu", w_fu, [P, 2, 2, P])  # [ki][kih, ko, koh]
            t_c0c = wtile("t_c0c", c0c[:], [P, 2], F32)
            t_wep = wtile("t_wep", w_ep[:], [8, H])
            t_bepc = wtile("t_bepc", bepc[:], [P, 2], F32)
            t_wepC = wtile("t_wepC", w_epC[:], [8, H])
            t_bepC = wtile("t_bepC", bepC[:], [1, H])
            t_wEB = [wtile(f"t_wEB{l}", wEB[l], [P, 2, 2 * H])
                     for l in range(L)]
            t_ebb = [wtile(f"t_ebb{l}", ebb[l], [1, 2 * H]) for l in range(L)]
            t_wDA = [wtile(f"t_wDA{l}", wDA[l], [P, 2, 2 * H])
                     for l in range(L)]
            t_dab = [wtile(f"t_dab{l}", dab[l], [1, 2 * H]) for l in range(L)]
            t_wC = [wtile(f"t_wC{l}", wC[l], [P, 2, H]) for l in range(L)]
            t_cbr = [wtile(f"t_cbr{l}", cbr[l], [1, H]) for l in range(L)]
            t_wPQ = wtile("t_wPQ", wPQ, [P, 2, 2 * H])
            t_w1c8 = wtile("t_w1c8", w1c8[:], [8, H])
            t_bd1 = wtile("t_bd1", bd1[:], [1, H])
            t_w2r = wtile("t_w2r", w2r[:], [1, H])

            mi_sb = wtile("mi_sb", mi_d[:], [P, nch4], I32)
            mdst_sb = wtile("mdst_sb", mdst_d[:], [P, nch4], F32)
            mval_sb = wtile("mval_sb", mval_d[:], [P, nch4], BF16)
            nmask_sb = wtile("nmask_sb", nmask_d[:], [P, NT], BF16)

            xg_d = dr.tile([NT, P, H], BF16, tag="xg_d")
            out_sb = wp.tile([P, nch], F32, tag="out_sb", name="out_sb")

            ones1 = wp.tile([1, P], BF16, tag="ones1", name="ones1")
            nc.gpsimd.memset(ones1[:], 1.0)
            io_i = wp.tile([P, P], I32, tag="io_i", name="io_i")
            nc.gpsimd.iota(io_i[:], pattern=[[1, P]], base=0,
                           channel_multiplier=0)
            io_ff = wp.tile([P, P], F32, tag="io_ff", name="io_ff")
            nc.vector.tensor_copy(io_ff[:], io_i[:])
            # batched iota [P, 4, P]: io4[p, j, q] = q
            io4_i = wp.tile([P, 4, P], I32, tag="io4_i", name="io4_i")
            nc.gpsimd.iota(io4_i[:], pattern=[[0, 4], [1, P]], base=0,
                           channel_multiplier=0)
            io4_f = wp.tile([P, 4, P], F32, tag="io4_f", name="io4_f")
            nc.vector.tensor_copy(io4_f[:], io4_i[:])
            iop_i = wp.tile([P, P], I32, tag="iop_i", name="iop_i")
            nc.gpsimd.iota(iop_i[:], pattern=[[0, P]], base=0,
                           channel_multiplier=1)
            iop_f = wp.tile([P, P], F32, tag="iop_f", name="iop_f")
            nc.vector.tensor_copy(iop_f[:], iop_i[:])
            ident = wp.tile([P, P], F32, tag="ident", name="ident")
            nc.vector.tensor_tensor(out=ident[:], in0=iop_f[:], in1=io_ff[:],
                                    op=ALU.is_equal)
            ident_b = wp.tile([P, P], BF16, tag="ident_b", name="ident_b")
            nc.vector.tensor_copy(ident_b[:], ident[:])

            # w2 broadcast tile
            w2r_b = wp.tile([1, H], BF16, tag="w2r_b", name="w2r_b")
            nc.gpsimd.dma_start(out=w2r_b[:], in_=w2r[:])
            pmW = ps.tile([P, 2 * H], F32, tag="bf32", name="pmw", bufs=2)
            nc.tensor.matmul(out=pmW[:, 0:H], lhsT=ones1[:], rhs=w2r_b[:],
                             start=True, stop=True)
            w2bc = wp.tile([P, H], BF16, tag="w2bc", name="w2bc")
            nc.scalar.activation(w2bc[:], pmW[:, 0:H], AF.Copy)

            def bias_acc(pm, row_bf, stop=True):
                nc.tensor.matmul(out=pm[:], lhsT=ones1[:, :pm.shape[0]],
                                 rhs=row_bf[:], start=False, stop=stop)

            # row [1,128-slice] f32 -> col [128,1] slice of dst
            def row2col(row_ap, dst_col, tag):
                pmS = ps.tile([P, 2 * H], F32, tag="bf32", name=f"pmS_{tag}",
                              bufs=2)
                nc.tensor.transpose(pmS[:, 0:1], row_ap, ident[:1, :1])
                nc.scalar.activation(dst_col, pmS[:, 0:1], AF.Copy)

            # ---------------- Phase 0a: fusion (x0, transposed) -------------
            NSN = (NPAD + 511) // 512     # 512-node supertiles (last ragged)
            for s in range(NSN):
                n0 = s * 512
                nn = min(512, NPAD - n0)
                rh = sb.tile([P, 2, 512], BF16, tag="rh", name="rh", bufs=2)
                nc.sync.dma_start(out=rh[:, 0, :nn], in_=ht[0, :, n0:n0 + nn])
                nc.scalar.dma_start(out=rh[:, 1, :nn], in_=ht[1, :, n0:n0 + nn])
                x0t = sb.tile([P, 2, 512], F32, tag="x0t", name="x0t", bufs=2)
                for fo in range(2):
                    pm = ps.tile([P, 2 * H], F32, tag="bf32", name="pm_fu")
                    for ki in range(2):
                        nc.tensor.matmul(out=pm[:, :nn],
                                         lhsT=t_wfu[:, ki, fo, :],
                                         rhs=rh[:, ki, :nn],
                                         start=(ki == 0), stop=(ki == 1))
                    nc.scalar.activation(x0t[:, fo, :nn], pm[:, :nn], AF.Relu,
                                         bias=t_c0c[:, fo:fo + 1])
                nc.sync.dma_start(
                    out=x_T[:, :, n0:n0 + nn].rearrange("a b c -> b a c"),
                    in_=x0t[:, :, :nn])
                xb = sb.tile([P, 2, 512], BF16, tag="xb", name="xb", bufs=2)
                nc.vector.tensor_copy(xb[:, :, :nn], x0t[:, :, :nn])
                # node matmuls for layer 0 tables
                for j in range((nn + P - 1) // P):
                    r0 = n0 + j * P
                    pmG = ps.tile([P, 2 * H], F32, tag="bf32", name="pmG")
                    for h in range(2):
                        nc.tensor.matmul(out=pmG[:],
                                         lhsT=xb[:, h, j * P:(j + 1) * P],
                                         rhs=t_wEB[0][:, h, :],
                                         start=(h == 0),
                                         stop=(zb and h == 1))
                    if not zb:
                        bias_acc(pmG, t_ebb[0])
                    gt = sb.tile([P, 2 * H], BF16, tag="gt", name="gt")
                    nc.vector.tensor_copy(gt[:], pmG[:])
                    nc.scalar.dma_start(out=gEB_loc[r0:r0 + P, :], in_=gt[:])
                    pmA = ps.tile([P, 2 * H], F32, tag="bf32", name="pmA")
                    for h in range(2):
                        nc.tensor.matmul(out=pmA[:],
                                         lhsT=xb[:, h, j * P:(j + 1) * P],
                                         rhs=t_wDA[0][:, h, :],
                                         start=(h == 0),
                                         stop=(zb and h == 1))
                    if not zb:
                        bias_acc(pmA, t_dab[0])
                    dat = sb.tile([P, 2 * H], BF16, tag="dat", name="dat")
                    nc.scalar.activation(dat[:], pmA[:], AF.Copy)
                    nc.sync.dma_start(out=gDA[r0:r0 + P, :], in_=dat[:])
            if DEBUG_TAPS:
                nc.sync.dma_start(out=taps["tap_x1"][:], in_=x_T[:])

            # AllGather layer-0 tables
            nc.gpsimd.collective_compute(
                "AllGather", ALU.bypass, replica_groups=[core_ids],
                ins=[gEB_loc.opt()], outs=[gEB_ag[0].opt()])

            # ---------------- Phase 0b: eproj (e0T + Ce0) ----------------
            for s in range(NSC):
                c0 = s * 512
                alt = sb.tile([8, 512], BF16, tag="alt", name="alt")
                nc.sync.dma_start(out=alt[:], in_=al_t[:, c0:c0 + 512])
                e0t = sb.tile([P, 2, 512], BF16, tag="e0t", name="e0t", bufs=2)
                for h in range(2):
                    pm = ps.tile([P, 2 * H], F32, tag="bf32", name="pm_ep")
                    nc.tensor.matmul(out=pm[:], lhsT=t_wep[:, h * P:(h + 1) * P],
                                     rhs=alt[:], start=True, stop=True)
                    nc.scalar.activation(e0t[:, h, :], pm[:], AF.Identity,
                                         bias=t_bepc[:, h:h + 1])
                nc.sync.dma_start(
                    out=e_T[:, :, c0:c0 + 512].rearrange("a b c -> b a c"),
                    in_=e0t[:])
                cet = sb.tile([P, 4, H], BF16, tag="cet", name="cet", bufs=2)
                for j in range(4):
                    pm = ps.tile([P, 2 * H], F32, tag="bf32", name="pm_ce0")
                    nc.tensor.matmul(out=pm[:, 0:H],
                                     lhsT=alt[:, j * P:(j + 1) * P],
                                     rhs=t_wepC[:], start=True, stop=zb)
                    if not zb:
                        nc.tensor.matmul(out=pm[:, 0:H], lhsT=ones1[:],
                                         rhs=t_bepC[:], start=False,
                                         stop=True)
                    nc.vector.tensor_copy(cet[:, j, :], pm[:, 0:H])
                nc.gpsimd.dma_start(
                    out=ce_b[c0:c0 + 512, :].rearrange("(a b) c -> b a c", a=4),
                    in_=cet[:])
            if DEBUG_TAPS:
                nc.sync.dma_start(out=taps["tap_e1"][:], in_=e_T[:])

            # ---------------- Layers ----------------
            for l in range(L):
                last = (l == L - 1)

                # (a) edge phase, 512-edge superchunks
                stp = ps.tile([33, 2 * H], F32, tag="stp", name="stp",
                              bufs=1)
                st_x = stp[0:1, :]
                st_e = stp[32:33, :]
                g_tiles = {}

                def issue_g4(s, lcur=l):
                    if s >= NS4:
                        return
                    gg = sb.tile([P, 4, 2 * H], BF16, tag="g", name="g",
                                 bufs=3)
                    for j in range(4):
                        nc.gpsimd.indirect_dma_start(
                            out=gg[:, j, :], out_offset=None,
                            in_=gEB_ag[lcur][:],
                            in_offset=bass.IndirectOffsetOnAxis(
                                ap=mi_sb[:, 4 * s + j:4 * s + j + 1],
                                axis=0))
                    g_tiles[s] = gg

                PF4 = 2
                for s in range(PF4):
                    issue_g4(s)

                dxa_of = {}       # tile -> dxa sbuf tile
                seg_of = {}       # tile -> pm_seg psum tile
                stash = {}        # superchunk -> (g4, ce4, sel4, E4)

                def do_A(s):
                    issue_g4(s + PF4)
                    c0 = s * 512
                    ce4 = sb.tile([P, 4, H], BF16, tag="ce", name="ce",
                                  bufs=2)
                    nc.sync.dma_start(
                        out=ce4[:],
                        in_=ce_b[c0:c0 + 512, :].rearrange(
                            "(a b) c -> b a c", a=4))
                    sT4 = sb.tile([P, 4 * P], BF16, tag="sT", name="sT",
                                  bufs=2)
                    nc.scalar.dma_start(out=sT4[:],
                                        in_=selT_d[:, c0:c0 + 512])
                    sel4 = sb.tile([P, 4, P], BF16, tag="sel", name="sel",
                                   bufs=3)
                    nc.vector.tensor_tensor(
                        out=sel4[:],
                        in0=mdst_sb[:, 4 * s:4 * s + 4].to_broadcast(
                            [P, 4, P]),
                        in1=io4_f[:], op=ALU.is_equal)
                    pmd2 = [ps.tile([P, 2, H], F32, tag="pd4",
                                    name=f"pmd2{half}", bufs=3)
                            for half in range(2)]
                    for j in range(4):
                        ch = 4 * s + j
                        if ch < nch:
                            t, k, K = sched[ch]
                            if k == 0:
                                dxa = sb.tile([P, 2 * H], BF16, tag="dxa",
                                              name="dxa", bufs=3)
                                nc.sync.dma_start(out=dxa[:],
                                                  in_=gDA[t * P:(t + 1) * P, :])
                                dxa_of[t] = dxa
                            rhs = dxa_of[t]
                        else:
                            rhs = dxa_of[max(dxa_of)]
                        nc.tensor.matmul(out=pmd2[j // 2][:, j % 2, :],
                                         lhsT=sT4[:, j * P:(j + 1) * P],
                                         rhs=rhs[:, 0:H],
                                         start=True, stop=True)
                    return (s, ce4, sel4, pmd2)

                def do_B(ab):
                    s, ce4, sel4, pmd2 = ab
                    c0 = s * 512
                    g4 = g_tiles.pop(s)
                    E4 = sb.tile([P, 4, 4 * H], BF16, tag="E4", name="E4",
                                 bufs=2)
                    # e_hat = Dx[dst] + Ce + Ex[src]
                    for half in range(2):
                        nc.vector.tensor_tensor(
                            out=E4[:, 2 * half:2 * half + 2, 3 * H:4 * H],
                            in0=pmd2[half][:],
                            in1=ce4[:, 2 * half:2 * half + 2, :],
                            op=ALU.add)
                    nc.vector.tensor_tensor(out=E4[:, :, 2 * H:3 * H],
                                            in0=E4[:, :, 3 * H:4 * H],
                                            in1=g4[:, :, 0:H], op=ALU.add)
                    if not last:
                        nc.sync.dma_start(
                            out=eh_b[c0:c0 + 512, :].rearrange(
                                "(a b) c -> b a c", a=4),
                            in_=E4[:, :, 2 * H:3 * H])
                    nc.scalar.activation(E4[:, :, H:2 * H],
                                         E4[:, :, 2 * H:3 * H], AF.Sigmoid)
                    nc.vector.tensor_tensor(out=E4[:, :, 0:H],
                                            in0=E4[:, :, H:2 * H],
                                            in1=g4[:, :, H:2 * H],
                                            op=ALU.mult)
                    if not last:
                        nc.scalar.activation(E4[:, :, 3 * H:4 * H],
                                             E4[:, :, 2 * H:3 * H],
                                             AF.Square)
                    stash[s] = (sel4, E4)

                def do_C(s):
                    sel4, E4 = stash.pop(s)
                    for j in range(4):
                        ch = 4 * s + j
                        if ch >= nch:
                            continue
                        t, k, K = sched[ch]
                        if k == 0:
                            seg_of[t] = ps.tile([P, 2 * H], F32, tag="bf32",
                                                name="pm_seg", bufs=2)
                        nc.tensor.matmul(out=seg_of[t][:],
                                         lhsT=sel4[:, j, :],
                                         rhs=E4[:, j, 0:2 * H],
                                         start=(k == 0), stop=(k == K - 1))
                        if not last:
                            nc.tensor.matmul(out=st_e[:],
                                             lhsT=mval_sb[:, ch:ch + 1],
                                             rhs=E4[:, j, 2 * H:4 * H],
                                             start=(ch == 0),
                                             stop=(ch == nch - 1),
                                             skip_group_check=True)
                        if k == K - 1:
                            # x_agg for tile t
                            pm_seg = seg_of.pop(t)
                            dxa = dxa_of.pop(t)
                            den = sb.tile([P, H], F32, tag="den", name="den",
                                          bufs=2)
                            nc.vector.tensor_scalar_add(den[:],
                                                        pm_seg[:, H:2 * H],
                                                        AGG_EPS)
                            rcp = sb.tile([P, H], F32, tag="rcp",
                                          name="rcp", bufs=2)
                            nc.vector.reciprocal(rcp[:], den[:])
                            d1 = sb.tile([P, H], F32, tag="d1", name="d1",
                                         bufs=2)
                            nc.vector.tensor_tensor(out=d1[:],
                                                    in0=pm_seg[:, 0:H],
                                                    in1=rcp[:],
                                                    op=ALU.mult)
                            stx = sb.tile([P, 2 * H], BF16, tag="stx",
                                          name="stx", bufs=2)
                            nc.gpsimd.tensor_tensor(out=stx[:, 0:H],
                                                    in0=d1[:],
                                                    in1=dxa[:, H:2 * H],
                                                    op=ALU.add)
                            nc.sync.dma_start(out=xg_d[t],
                                              in_=stx[:, 0:H])
                            nc.vector.tensor_tensor(out=stx[:, H:2 * H],
                                                    in0=stx[:, 0:H],
                                                    in1=stx[:, 0:H],
                                                    op=ALU.mult)
                            nc.tensor.matmul(out=st_x[:],
                                             lhsT=nmask_sb[:, t:t + 1],
                                             rhs=stx[:], start=(t == 0),
                                             stop=(t == NT - 1),
                                             skip_group_check=True)

                ab_prev = None
                for s in range(NS4):
                    ab = do_A(s)
                    if ab_prev is not None:
                        do_C(ab_prev[0])
                    do_B(ab)
                    ab_prev = ab
                do_C(ab_prev[0])

                # (b) stats AllReduce
                stc = sb.tile([1, 4 * H], F32, tag="stc", name="stc", bufs=1)
                nc.vector.tensor_copy(stc[:, 0:2 * H], st_x[:])
                if not last:
                    nc.vector.tensor_copy(stc[:, 2 * H:4 * H], st_e[:])
                else:
                    nc.gpsimd.memset(stc[:, 2 * H:4 * H], 0.0)
                nc.sync.dma_start(out=st_i[:], in_=stc[:])
                nc.gpsimd.collective_compute(
                    "AllReduce", ALU.add, replica_groups=[core_ids],
                    ins=[st_i.opt()], outs=[st_o_l[l].opt()])
                stg = sb.tile([1, 4 * H], F32, tag="stg", name="stg", bufs=1)
                nc.sync.dma_start(out=stg[:], in_=st_o_l[l][:])

                # (c) BN s,t as per-partition columns
                def bn_cols(sl, cnt, g_ap, b_ap, nm_):
                    mu = sb.tile([1, H], F32, tag="mu", name=f"mu{nm_}",
                                 bufs=1)
                    nc.scalar.mul(mu[:], stg[:, sl:sl + H], 1.0 / cnt)
                    m2 = sb.tile([1, H], F32, tag="m2", name=f"m2{nm_}",
                                 bufs=1)
                    nc.scalar.mul(m2[:], stg[:, sl + H:sl + 2 * H], 1.0 / cnt)
                    var = sb.tile([1, H], F32, tag="var",
                                  name=f"var{nm_}", bufs=1)
                    nc.vector.tensor_tensor(out=var[:], in0=mu[:], in1=mu[:],
                                            op=ALU.mult)
                    nc.vector.tensor_tensor(out=var[:], in0=m2[:], in1=var[:],
                                            op=ALU.subtract)
                    nc.vector.tensor_scalar_add(var[:], var[:], BN_EPS)
                    sd = sb.tile([1, H], F32, tag="sd", name=f"sd{nm_}",
                                 bufs=1)
                    nc.scalar.activation(sd[:], var[:], AF.Sqrt)
                    rs = sb.tile([1, H], F32, tag="rs", name=f"rs{nm_}",
                                 bufs=1)
                    nc.vector.reciprocal(rs[:], sd[:])
                    gg = sb.tile([1, H], F32, tag="gg", name=f"gg{nm_}",
                                 bufs=1)
                    nc.sync.dma_start(out=gg[:], in_=g_ap)
                    bb = sb.tile([1, H], F32, tag="bb", name=f"bb{nm_}",
                                 bufs=1)
                    nc.scalar.dma_start(out=bb[:], in_=b_ap)
                    srow = sb.tile([1, H], F32, tag="sr",
                                   name=f"sr{nm_}", bufs=1)
                    nc.vector.tensor_tensor(out=srow[:], in0=gg[:], in1=rs[:],
                                            op=ALU.mult)
                    trow = sb.tile([1, H], F32, tag="tr",
                                   name=f"tr{nm_}", bufs=1)
                    nc.vector.tensor_tensor(out=trow[:], in0=mu[:],
                                            in1=srow[:], op=ALU.mult)
                    nc.vector.tensor_tensor(out=trow[:], in0=bb[:],
                                            in1=trow[:], op=ALU.subtract)
                    scol = sb.tile([P, 2], F32, tag=f"sc{nm_}",
                                   name=f"sc{nm_}", bufs=1)
                    tcol = sb.tile([P, 2], F32, tag=f"tc{nm_}",
                                   name=f"tc{nm_}", bufs=1)
                    for h in range(2):
                        row2col(srow[:, h * P:(h + 1) * P], scol[:, h:h + 1],
                                f"s{nm_}{h}")
                        row2col(trow[:, h * P:(h + 1) * P], tcol[:, h:h + 1],
                                f"t{nm_}{h}")
                    return scol, tcol

                sxc, txc = bn_cols(0, N, gxg[l], gxb[l], "x")
                if not last:
                    sec, tec = bn_cols(2 * H, E, geg[l], geb[l], "e")

                # (d) pass-2 x fused with next-layer node matmuls
                for t in range(NT):
                    r0 = t * P
                    xgt = sb.tile([P, H], BF16, tag="xgt", name="xgt",
                                  bufs=3)
                    nc.sync.dma_start(out=xgt[:], in_=xg_d[t])
                    pmT = ps.tile([P, 2 * H], BF16, tag="pb16", name="pmT")
                    for h in range(2):
                        nc.tensor.transpose(
                            pmT[:, h * P:(h + 1) * P],
                            xgt[:, h * P:(h + 1) * P],
                            ident_b[:])
                    xbn = sb.tile([P, 2, P], BF16, tag="xbn", name="xbn")
                    for h in range(2):
                        nc.scalar.activation(xbn[:, h, :],
                                             pmT[:, h * P:(h + 1) * P],
                                             AF.Relu, bias=txc[:, h:h + 1],
                                             scale=sxc[:, h:h + 1])
                    xoT = sb.tile([P, 2, P], F32, tag="xoT", name="xoT")
                    nc.scalar.dma_start(
                        out=xoT[:],
                        in_=x_T[:, :, r0:r0 + P].rearrange("a b c -> b a c"))
                    xnT = sb.tile([P, 2, P], F32, tag="xnT", name="xnT")
                    nc.vector.tensor_tensor(out=xnT[:], in0=xoT[:],
                                            in1=xbn[:], op=ALU.add)
                    if not last or DEBUG_TAPS:
                        nc.sync.dma_start(
                            out=x_T[:, :, r0:r0 + P].rearrange(
                                "a b c -> b a c"),
                            in_=xnT[:])
                    lhx = sb.tile([P, 2, P], BF16, tag="lhx", name="lhx")
                    nc.vector.tensor_copy(lhx[:], xnT[:])
                    if not last:
                        pmG = ps.tile([P, 2 * H], F32, tag="bf32", name="pmG2")
                        for h in range(2):
                            nc.tensor.matmul(out=pmG[:], lhsT=lhx[:, h, :],
                                             rhs=t_wEB[l + 1][:, h, :],
                                             start=(h == 0),
                                             stop=(zb and h == 1))
                        if not zb:
                            bias_acc(pmG, t_ebb[l + 1])
                        gt = sb.tile([P, 2 * H], BF16, tag="gt", name="gt2")
                        nc.vector.tensor_copy(gt[:], pmG[:])
                        nc.gpsimd.dma_start(out=gEB_loc[r0:r0 + P, :],
                                            in_=gt[:])
                        pmA = ps.tile([P, 2 * H], F32, tag="bf32", name="pmA2")
                        for h in range(2):
                            nc.tensor.matmul(out=pmA[:], lhsT=lhx[:, h, :],
                                             rhs=t_wDA[l + 1][:, h, :],
                                             start=(h == 0),
                                             stop=(zb and h == 1))
                        if not zb:
                            bias_acc(pmA, t_dab[l + 1])
                        dat = sb.tile([P, 2 * H], BF16, tag="dat",
                                      name="dat2")
                        nc.scalar.activation(dat[:], pmA[:], AF.Copy)
                        nc.sync.dma_start(out=gDA[r0:r0 + P, :], in_=dat[:])
                    else:
                        pmG = ps.tile([P, 2 * H], F32, tag="bf32", name="pmPQ")
                        for h in range(2):
                            nc.tensor.matmul(out=pmG[:], lhsT=lhx[:, h, :],
                                             rhs=t_wPQ[:, h, :],
                                             start=(h == 0), stop=(h == 1))
                        gt = sb.tile([P, 2 * H], BF16, tag="gt", name="gtPQ")
                        nc.vector.tensor_copy(gt[:], pmG[:])
                        nc.scalar.dma_start(out=p_loc[r0:r0 + P, :],
                                            in_=gt[:, 0:H])
                        nc.sync.dma_start(out=q_loc[r0:r0 + P, :],
                                          in_=gt[:, H:2 * H])
                if DEBUG_TAPS:
                    nc.sync.dma_start(out=taps[f"tap_x{l + 2}"][:], in_=x_T[:])

                # (e) AllGather next tables (overlaps pass-2 e)
                if not last:
                    nc.gpsimd.collective_compute(
                        "AllGather", ALU.bypass, replica_groups=[core_ids],
                        ins=[gEB_loc.opt()], outs=[gEB_ag[l + 1].opt()])
                else:
                    nc.gpsimd.collective_compute(
                        "AllGather", ALU.bypass, replica_groups=[core_ids],
                        ins=[p_loc.opt()], outs=[p_ag.opt()])

                # (f) pass-2 e fused with next-layer Ce
                if not last:
                    u = 0
                    while u * 2 < nch:
                        w = min(2, nch - u * 2)
                        c0 = u * 2 * P
                        ww = w * P
                        ea = sb.tile([P, 2, H], BF16, tag="ea", name="ea", bufs=2)
                        nc.sync.dma_start(
                            out=ea[:, :w, :],
                            in_=eh_b[c0:c0 + ww, :].rearrange(
                                "(a b) c -> b a c", a=w))
                        pmT2 = ps.tile([P, 2 * H], BF16, tag="pb16",
                                       name="pmT2")
                        for h in range(2):
                            for j in range(w):
                                nc.tensor.transpose(
                                    pmT2[:, h * ww + j * P:h * ww + (j + 1) * P],
                                    ea[:, j, h * P:(h + 1) * P], ident_b[:])
                        ebn = sb.tile([P, 2, 2 * P], BF16, tag="ebn",
                                      name="ebn")
                        for h in range(2):
                            nc.scalar.activation(ebn[:, h, :ww],
                                                 pmT2[:, h * ww:h * ww + ww],
                                                 AF.Relu,
                                                 bias=tec[:, h:h + 1],
                                                 scale=sec[:, h:h + 1])
                        eoT = sb.tile([P, 2, 2 * P], BF16, tag="eoT",
                                      name="eoT", bufs=2)
                        nc.scalar.dma_start(
                            out=eoT[:, :, :ww],
                            in_=e_T[:, :, c0:c0 + ww].rearrange(
                                "a b c -> b a c"))
                        enT = sb.tile([P, 2, 2 * P], BF16, tag="enT",
                                      name="enT")
                        nc.vector.tensor_tensor(out=enT[:, :, :ww],
                                                in0=eoT[:, :, :ww],
                                                in1=ebn[:, :, :ww],
                                                op=ALU.add)
                        nc.sync.dma_start(
                            out=e_T[:, :, c0:c0 + ww].rearrange(
                                "a b c -> b a c"),
                            in_=enT[:, :, :ww])
                        cet = sb.tile([P, 2, H], BF16, tag="cet2",
                                      name="cet2")
                        for j in range(w):
                            pm = ps.tile([P, 2 * H], F32, tag="bf32",
                                         name="pmCe")
                            for h in range(2):
                                nc.tensor.matmul(out=pm[:, 0:H],
                                                 lhsT=enT[:, h,
                                                          j * P:(j + 1) * P],
                                                 rhs=t_wC[l + 1][:, h, :],
                                                 start=(h == 0),
                                                 stop=(zb and h == 1))
                            if not zb:
                                nc.tensor.matmul(out=pm[:, 0:H],
                                                 lhsT=ones1[:],
                                                 rhs=t_cbr[l + 1][:],
                                                 start=False, stop=True)
                            nc.vector.tensor_copy(cet[:, j, :], pm[:, 0:H])
                        nc.gpsimd.dma_start(
                            out=ce_b[c0:c0 + ww, :].rearrange(
                                "(a b) c -> b a c", a=w),
                            in_=cet[:, :w, :])
                        u += 1

            # ---------------- Decoder ----------------
            # R_e = aligned @ W1c + dec1_b  (overlaps AllGather of P)
            for s in range(NSC):
                c0 = s * 512
                alt = sb.tile([8, 512], BF16, tag="alt", name="alt_d")
                nc.sync.dma_start(out=alt[:], in_=al_t[:, c0:c0 + 512])
                ret = sb.tile([P, 4, H], BF16, tag="ret", name="ret", bufs=2)
                for j in range(4):
                    pm = ps.tile([P, 2 * H], F32, tag="bf32", name="pm_re")
                    nc.tensor.matmul(out=pm[:, 0:H],
                                     lhsT=alt[:, j * P:(j + 1) * P],
                                     rhs=t_w1c8[:], start=True, stop=zb)
                    if not zb:
                        nc.tensor.matmul(out=pm[:, 0:H], lhsT=ones1[:],
                                         rhs=t_bd1[:], start=False, stop=True)
                    nc.vector.tensor_copy(ret[:, j, :], pm[:, 0:H])
                nc.gpsimd.dma_start(
                    out=ce_b[c0:c0 + 512, :].rearrange("(a b) c -> b a c",
                                                       a=4),
                    in_=ret[:])

            if DEBUG_TAPS:
                nc.sync.dma_start(out=taps["tap_p"][:], in_=p_loc[:])
                nc.sync.dma_start(out=taps["tap_q"][:], in_=q_loc[:])
                nc.sync.dma_start(out=taps["tap_re"][:], in_=ce_b[:])
            pg_tiles = {}

            def issue_pg4(s):
                if s >= NS4:
                    return
                gg = sb.tile([P, 4, H], BF16, tag="pg", name="pg", bufs=3)
                for j in range(4):
                    nc.gpsimd.indirect_dma_start(
                        out=gg[:, j, :], out_offset=None, in_=p_ag[:],
                        in_offset=bass.IndirectOffsetOnAxis(
                            ap=mi_sb[:, 4 * s + j:4 * s + j + 1], axis=0))
                pg_tiles[s] = gg

            PF4 = 2
            for s in range(PF4):
                issue_pg4(s)
            qt_of = {}
            for s in range(NS4):
                issue_pg4(s + PF4)
                pg4 = pg_tiles.pop(s)
                c0 = s * 512
                re4 = sb.tile([P, 4, H], BF16, tag="re", name="re", bufs=2)
                nc.sync.dma_start(
                    out=re4[:],
                    in_=ce_b[c0:c0 + 512, :].rearrange("(a b) c -> b a c",
                                                       a=4))
                sT4 = sb.tile([P, 4 * P], BF16, tag="sT", name="sTd",
                              bufs=2)
                nc.scalar.dma_start(out=sT4[:], in_=selT_d[:, c0:c0 + 512])
                pmq2 = [ps.tile([P, 2, H], F32, tag="pd4",
                                name=f"pmq2{half}", bufs=3)
                        for half in range(2)]
                for j in range(4):
                    ch = 4 * s + j
                    if ch < nch:
                        t, k, K = sched[ch]
                        if k == 0:
                            qt = sb.tile([P, H], BF16, tag="qt", name="qt",
                                         bufs=3)
                            nc.sync.dma_start(out=qt[:],
                                              in_=q_loc[t * P:(t + 1) * P, :])
                            qt_of[t] = qt
                        rhs = qt_of[t]
                    else:
                        rhs = qt_of[max(qt_of)]
                    nc.tensor.matmul(out=pmq2[j // 2][:, j % 2, :],
                                     lhsT=sT4[:, j * P:(j + 1) * P],
                                     rhs=rhs[:], start=True, stop=True)
                h14 = sb.tile([P, 4, H], BF16, tag="h1", name="h1", bufs=2)
                nc.vector.tensor_tensor(out=h14[:], in0=pg4[:], in1=re4[:],
                                        op=ALU.add)
                h24 = sb.tile([P, 4, H], BF16, tag="h2", name="h2", bufs=2)
                for half in range(2):
                    nc.vector.tensor_tensor(
                        out=h24[:, 2 * half:2 * half + 2, :],
                        in0=pmq2[half][:],
                        in1=h14[:, 2 * half:2 * half + 2, :], op=ALU.add)
                nc.scalar.activation(h24[:], h24[:], AF.Relu)
                for j in range(4):
                    ch = 4 * s + j
                    if ch >= nch:
                        continue
                    scr = sb.tile([P, H], F32, tag="scr", name="scr",
                                  bufs=2)
                    nc.vector.tensor_tensor(out=scr[:], in0=h24[:, j, :],
                                            in1=w2bc[:], op=ALU.mult)
                    nc.vector.tensor_reduce(out=out_sb[:, ch:ch + 1],
                                            in_=scr[:], op=ALU.add,
                                            axis=mybir.AxisListType.X)
            nc.sync.dma_start(out=out_d[:], in_=out_sb[:])

    _split_excess_waits(nc, mybir)
    return nc


def _split_excess_waits(nc, mybir, max_waits=1):
    """walrus in this env accepts max 1 sem wait per instruction: spill
    extras onto same-engine nops placed before the instruction."""
    for f in nc.m.functions:
        for bb in f.blocks:
            insts = list(bb.instructions)
            out_l = []
            for inst in insts:
                si = inst.sync_info
                waits = list(si.on_wait) if (si and si.on_wait) else []
                if len(waits) > max_waits:
                    extra = waits[:-max_waits]
                    keep = waits[-max_waits:]
                    for i in range(0, len(extra), max_waits):
                        nop = mybir.InstNoOp(
                            name=nc.get_next_instruction_name(),
                            engine=inst.engine, ins=[], outs=[],
                            sync_info=mybir.SyncInfo(
                                on_wait=extra[i:i + max_waits], on_update=[]))
                        nc.register_instruction(nop)
                        out_l.append(nop)
                    del si.on_wait[:]
                    si.on_wait.extend(keep)
                out_l.append(inst)
            if len(out_l) != len(insts):
                bb.instructions = out_l


# ----------------------------------------------------------------- wrapper
_CACHE = {}


def kernel(**inputs):
    edge_index = np.asarray(inputs["edge_index_new"])
    aligned = np.asarray(inputs["aligned_features"], dtype=np.float32)
    h_old = np.asarray(inputs["h_nodes_old"], dtype=np.float32)
    assert int(inputs["num_nodes"]) == N

    prep = _host_prep(edge_index, aligned)
    nch, EC, ECP, WR = prep["nch"], prep["EC"], prep["ECP"], prep["WR"]

    fw = np.asarray(inputs["fusion_w"], np.float32)
    fb = np.asarray(inputs["fusion_b"], np.float32)
    c0 = fw[:H].sum(axis=0) + fb                     # [256]

    def g(nm):
        return np.asarray(inputs[nm], np.float32)

    wEB = np.stack([np.concatenate([g("E_w")[l], g("B_w")[l]], axis=1)
                    .reshape(2, P, 2 * H) for l in range(L)])
    ebb = np.stack([np.concatenate([g("E_b")[l], g("B_b")[l]])[None]
                    for l in range(L)])
    wDA = np.stack([np.concatenate([g("D_w")[l], g("A_w")[l]], axis=1)
                    .reshape(2, P, 2 * H) for l in range(L)])
    dab = np.stack([np.concatenate([g("D_b")[l], g("A_b")[l]])[None]
                    for l in range(L)])
    wC = np.stack([g("C_w")[l].reshape(2, P, H) for l in range(L)])
    cbr = np.stack([g("C_b")[l][None] for l in range(L)])

    ep_w = g("eproj_w")                              # [8,256]
    ep_b = g("eproj_b")
    w_epC = ep_w @ g("C_w")[0]                       # [8,256]
    bepC = (ep_b @ g("C_w")[0] + g("C_b")[0])[None]

    d1 = g("dec1_w")                                 # [520,256]
    d2 = g("dec2_w")                                 # [256,1]

    # fusion weights blocked: [ki, kih(128), ko, koh(128)]
    wfu = fw[H:].reshape(2, P, 2, P)

    zb = all(np.abs(g(nm)).max() == 0 for nm in
             ["A_b", "B_b", "C_b", "D_b", "E_b", "dec1_b"])
    key = (nch, zb) + tuple(prep["K_t"])
    if key not in _CACHE:
        _CACHE[key] = _build_nc(nch, prep["K_t"], ECP, WR, zb)
    nc = _CACHE[key]

    shared = {
        "w_fu": wfu, "c0c": c0.reshape(2, P).T,
        "w_ep": ep_w.astype(_bf16), "bepc": ep_b.reshape(2, P).T,
        "w_epC": w_epC.astype(_bf16), "bepC": bepC,
        "wEB": wEB, "ebb": ebb, "wDA": wDA, "dab": dab,
        "wC": wC, "cbr": cbr,
        "gxg": g("bn_x_g")[:, None, :], "gxb": g("bn_x_b")[:, None, :],
        "geg": g("bn_e_g")[:, None, :], "geb": g("bn_e_b")[:, None, :],
        "wPQ": np.concatenate([d1[:H], d1[H:2 * H]], axis=1).reshape(2, P, 2 * H),
        "w1c8": d1[2 * H:].astype(_bf16),
        "bd1": np.asarray(inputs["dec1_b"], np.float32)[None],
        "w2r": d2[:, 0][None],
        "nmask_d": prep["nmask"],
    }
    in_maps = []
    for c in range(NCORES):
        lo = c * NP_
        hT = np.zeros((2, P, NPAD), dtype=_bf16)
        hs = h_old[lo:lo + NP_].astype(_bf16)        # [12500, 256]
        hT[0, :, :NP_] = hs[:, :P].T
        hT[1, :, :NP_] = hs[:, P:].T
        m = dict(shared)
        m["ht"] = hT
        m["al_t"] = prep["al_t"][c]
        m["mi_d"] = prep["mi"][c]
        m["mdst_d"] = prep["mdst"][c]
        m["mval_d"] = prep["mval"][c]
        m["selT_d"] = prep["selT"][c]
        in_maps.append(m)

    from concourse.bass_utils import run_bass_kernel_spmd
    res = run_bass_kernel_spmd(nc, in_maps, list(range(NCORES)),
                               trace=PROFILE)
    if PROFILE and res.exec_time_ns is not None:
        print(f"HW exec time: {res.exec_time_ns} ns")

    allout = np.concatenate([np.asarray(res.results[c]["out_d"]).T.ravel()
                             for c in range(NCORES)])
    b2 = float(np.asarray(inputs["dec2_b"], np.float32).ravel()[0])
    flow = (allout[prep["perm"]] + b2).astype(np.float32)[:, None]
    if DEBUG_TAPS:
        kernel.taps = [
            {k: np.asarray(v) for k, v in r.items() if k.startswith("tap")}
            for r in res.results]
        kernel.prep = prep
    return flow
